# revision 22
# baseline (speedup 1.0000x reference)
"""Context-segment scoring kernel for Trainium2 (Bass/Tile).

Computes out[b, n] = sum_e c[b, n, e] * s[b, e] for
c = c_embeds [32, 32, 32, 8, 256] viewed as [B=32, N=8192, E=256] and
s = s_embeds [32, 256].

Sharding: data-parallel over batch — 8 NeuronCores, 4 batches each.
Per core: stream c (32 MiB) through SBUF in 2 MiB groups
([128 partitions x 16 rows x 256]) on the Sync-engine HWDGE ring,
loaded as halves (quarters at the pipeline edges) so completion
granularity is fine enough for buffer recycling to never gate a
straggling SDMA engine; cin lookahead is 11 groups (22 MiB).
The per-batch segment embeddings are broadcast on-chip: one 4 KiB DMA
stages s on a single partition, the idle TensorE multiplies it by a
ones[1,128] stationary to replicate it across partitions into PSUM,
and the DVE reads in1 straight from PSUM — no 128x-amplified HBM
broadcast reads. Reduce work splits 10:6 between DVE
(tensor_tensor_reduce rows, in place) and ScalarE (activation accum
reduces of a wide DVE product). Results accumulate in one resident
[128, 256] tile whose free-dim layout (b, g, j) makes store
descriptors contiguous per partition; batches store as they finish and
the final 8 KiB piece goes out on the idle Sync ring. The host
transposes the permuted [128, 256] DRAM image back to [4, 8192].
"""

import numpy as np

import concourse.bacc as bacc
import concourse.bass as bass
import concourse.mybir as mybir
import concourse.tile as tile
from concourse.bass_utils import run_bass_kernel_spmd

B, N, E = 32, 8192, 256
NCORES = 8
B_LOC = B // NCORES          # 4 batches per core
P = 128                      # SBUF partitions
ROWS = 16                    # n-rows per partition per group
GROUP_N = P * ROWS           # 2048 n per group
G = N // GROUP_N             # 4 groups per batch
NGROUPS = G * B_LOC          # 16 groups per core
# Per-group engine plan: 'A' = fused multiply+reduce rows on DVE (in-place,
# no product tile); 'S' = wide DVE multiplies (halves), ScalarE reduces the
# rows. GpSimd elementwise is NOT used: it share-locks the DVE SBUF port and
# was measured to slow every concurrent DVE op by ~36%.
# 'A' = per-row fused multiply+reduce on DVE (~6.6us/group, single pass at
# the DVE reduce rate); 'S' = wide DVE multiplies (halves, fast elementwise
# rate) + ScalarE activation-accum rows (~3.1us DVE + ~9.3us Scalar).
# 9A:7S balances DVE ~81us / Scalar ~77us against the ~83us DMA stream.
# ('R' whole-group tensor_reduce was measured WORSE: standalone reduce runs
# at ~1.7ns/elem, half the elementwise rate -> 10us/group all-DVE.)
PLAN = ["A", "S", "A", "S", "A", "S", "A", "S", "A", "S", "A", "S", "S", "A", "A", "A"]

# HW-validated flags (tensor_tensor_reduce in-place dies on HW; avoid)
USE_PE_BCAST = True   # s via TensorE broadcast into PSUM (False: HBM bcast DMA)
USE_SYNC_STORE = True  # final 8 KiB store on Sync ring (False: Scalar)

F32 = mybir.dt.float32


def build_body(tc, out_ap, c_ap, s_ap):
    """Trace the per-core Tile program. APs are DRAM access patterns:
    out [P, B_LOC*G*ROWS] (permuted; host untangles), c [B_LOC, N, E],
    s [B_LOC, E]."""
    nc = tc.nc
    with (
        tc.tile_pool(name="sstage", bufs=1) as sstage_pool,
        tc.tile_pool(name="ones", bufs=1) as ones_pool,
        tc.tile_pool(name="sps", bufs=1, space="PSUM") as sps_pool,
        tc.tile_pool(name="cpair", bufs=4) as cpair_pool,
        tc.tile_pool(name="cedge", bufs=2) as cedge_pool,
        tc.tile_pool(name="prod", bufs=2) as prod_pool,
        tc.tile_pool(name="res", bufs=1) as res_pool,
        tc.tile_pool(name="dump", bufs=2) as dump_pool,
    ):
        # Flat n view so load pairs can cross batch boundaries.
        c_flat = c_ap.rearrange("b n e -> (b n) e")

        # Loads: the Tile scheduler tracks DMA completion on only 8 sem
        # lanes per issuing engine, so at most 8 dma_starts are in flight
        # per ring. Middle groups therefore load as PAIRS (one 4 MiB
        # dispatch, 32 KiB/partition descriptors) so the window covers the
        # whole stream and the SBUF pool (4 pair bufs + edges ~ 10 groups)
        # is the only recycle gate. The first group loads as quarters on
        # the then-idle Scalar ring (DVE starts sooner, Sync's window is
        # spent on pairs); the last group as quarters on Sync so the
        # compute tail stays short.
        tiles = {}

        def load(gi):
            if gi in (0, NGROUPS - 1):
                ct = cedge_pool.tile([P, ROWS, E], F32, tag="cedge", name="ce")
                src = c_flat[gi * GROUP_N:(gi + 1) * GROUP_N, :].rearrange(
                    "(p j) e -> p j e", j=ROWS
                )
                eng = nc.scalar if gi == 0 else nc.sync
                C = ROWS // 4
                for q in range(4):
                    eng.dma_start(
                        ct[:, q * C:(q + 1) * C, :], src[:, q * C:(q + 1) * C, :]
                    )
                tiles[gi] = ct
            else:  # odd gi: load the (gi, gi+1) pair in one dispatch.
                # 4-D AP keeps each group's standard (p, j) layout: every
                # partition reads two 16 KiB chunks, one per group.
                cp = cpair_pool.tile([P, 2, ROWS, E], F32, tag="cpair", name="cp")
                src = c_flat[gi * GROUP_N:(gi + 2) * GROUP_N, :].rearrange(
                    "(gg p j) e -> p gg j e", gg=2, j=ROWS
                )
                nc.sync.dma_start(cp[:], src)
                tiles[gi] = cp[:, 0, :, :]
                tiles[gi + 1] = cp[:, 1, :, :]

        # --- on-chip segment-embedding broadcast -------------------------
        # One 4 KiB DMA lands all four batches' s on partition 0; TensorE
        # replicates each across 128 partitions into its own PSUM bank
        # (ones[1,128].T @ s[1,256]); ScalarE copies each into SBUF.
        if USE_PE_BCAST:
            s_stage = sstage_pool.tile([1, B_LOC * E], F32, tag="s_stage")
            nc.scalar.dma_start(
                s_stage[:, :], s_ap.rearrange("b e -> (b e)").unsqueeze(0)
            )
            # Group 0's quarters go out on Scalar right behind the tiny
            # s_stage load, BEFORE the PSUM->SBUF copies (which block on
            # the PE matmuls) so the c stream starts immediately.
            load(0)
            ones = ones_pool.tile([1, P], F32, tag="ones")
            nc.vector.memset(ones[:, :], 1.0)
            # one PSUM bank (512 f32) per batch so each matmul output is
            # bank-aligned
            s_ps = sps_pool.tile([P, B_LOC, 512], F32, tag="s_ps")
            for b in range(B_LOC):
                nc.tensor.matmul(
                    s_ps[:, b, 0:E],
                    ones[:, :],
                    s_stage[:, b * E:(b + 1) * E],
                    start=True,
                    stop=True,
                )
            # ScalarE copies each batch's broadcast PSUM->SBUF: DVE rows
            # read in1 from SBUF at full rate (PSUM in1 cost ~+25% per row).
            s_sb = ones_pool.tile([P, B_LOC * E], F32, tag="s_sb")
            for b in range(B_LOC):
                nc.scalar.copy(s_sb[:, b * E:(b + 1) * E], s_ps[:, b, 0:E])
            s_in1 = [s_sb[:, b * E:(b + 1) * E] for b in range(B_LOC)]
        else:
            sb_all = sstage_pool.tile([P, B_LOC * E], F32, tag="s_sb")
            load(0)
            for b in range(B_LOC):
                s_src = s_ap[b, :].unsqueeze(0).broadcast_to([P, E])
                nc.scalar.dma_start(sb_all[:, b * E:(b + 1) * E], s_src)
            s_in1 = [sb_all[:, b * E:(b + 1) * E] for b in range(B_LOC)]

        # All per-row results accumulate in one SBUF tile; free-dim order
        # (b, g, j) keeps each store's per-partition bytes contiguous.
        res_all = res_pool.tile([P, B_LOC, G, ROWS], F32, tag="res")

        HALF = ROWS // 2

        for b in range(B_LOC):
            for g in range(G):
                gi = b * G + g
                if gi not in tiles:
                    load(gi)
                ct = tiles.pop(gi)

                res = res_all[:, b, g, :]
                if PLAN[gi] == "A":
                    # Fused multiply+reduce per row on DVE: fine-grained so
                    # the first group computes per-quarter as data lands and
                    # the last group's tail is short. In-place over ct.
                    for j in range(ROWS):
                        nc.vector.affine_mul_reduce(
                            out=ct[:, j, :],
                            accum_out=res[:, j:j + 1],
                            in0=ct[:, j, :],
                            in1=s_in1[b],
                            scale=1.0,
                            bias=0.0,
                        )
                elif PLAN[gi] == "R":
                    # Whole-group path, all DVE, two wide ops: in-place
                    # multiply, then one segmented reduce over the innermost
                    # axis ([P,16,256] -> [P,16]). Avoids the ~230 ns
                    # per-instruction overhead of 16 row ops.
                    s_bc = s_in1[b].unsqueeze(1).broadcast_to([P, ROWS, E])
                    nc.vector.tensor_tensor(
                        out=ct[:],
                        in0=ct[:],
                        in1=s_bc,
                        op=mybir.AluOpType.mult,
                    )
                    nc.vector.tensor_reduce(
                        out=res[:, :],
                        in_=ct[:],
                        axis=mybir.AxisListType.X,
                        op=mybir.AluOpType.add,
                    )
                else:
                    # Two wide DVE multiplies (halves, so the product pool
                    # recycles finely), then ScalarE reduces the rows.
                    for h in range(2):
                        pr = prod_pool.tile([P, HALF, E], F32, tag="prod", name="pr")
                        s_bc = s_in1[b].unsqueeze(1).broadcast_to([P, HALF, E])
                        nc.vector.tensor_tensor(
                            out=pr[:],
                            in0=ct[:, h * HALF:(h + 1) * HALF, :],
                            in1=s_bc,
                            op=mybir.AluOpType.mult,
                        )
                        dump = dump_pool.tile([P, E], F32, tag="dump", name="dump")
                        for j in range(HALF):
                            nc.scalar.activation(
                                dump[:, :],
                                pr[:, j, :],
                                mybir.ActivationFunctionType.Copy,
                                bias=0.0,
                                scale=1.0,
                                accum_out=res[:, h * HALF + j:h * HALF + j + 1],
                            )

            # Store finished results eagerly so only the last (8 KiB) piece
            # sits on the critical-path tail. res free-dim layout makes each
            # store's bytes contiguous per partition (>=256 B descriptors).
            fb = b * G * ROWS
            if b < B_LOC - 1:
                nc.scalar.dma_start(
                    out_ap[:, fb:fb + G * ROWS], res_all[:, b, :, :]
                )
        # Last batch: groups 0-2 store from the ScalarE ring as soon as
        # group 2 is done; the final group's 8 KiB goes out on the idle
        # Sync ring right after its last row completes.
        lb = (B_LOC - 1) * G * ROWS
        nc.scalar.dma_start(
            out_ap[:, lb:lb + 3 * ROWS], res_all[:, B_LOC - 1, 0:3, :]
        )
        eng = nc.sync if USE_SYNC_STORE else nc.scalar
        eng.dma_start(
            out_ap[:, lb + 3 * ROWS:lb + 4 * ROWS],
            res_all[:, B_LOC - 1, 3, :],
        )


_NC_CACHE = None


def _get_nc():
    global _NC_CACHE
    if _NC_CACHE is None:
        nc = bacc.Bacc(
            "TRN2",
            target_bir_lowering=False,
            debug=False,
            num_devices=NCORES,
        )
        c = nc.dram_tensor("c", [B_LOC, N, E], F32, kind="ExternalInput")
        s = nc.dram_tensor("s", [B_LOC, E], F32, kind="ExternalInput")
        o = nc.dram_tensor("o", [P, B_LOC * G * ROWS], F32, kind="ExternalOutput")
        with tile.TileContext(nc) as tc:
            build_body(tc, o.ap(), c.ap(), s.ap())
        nc.compile()
        _NC_CACHE = nc
    return _NC_CACHE


def _run(c_embeds: np.ndarray, s_embeds: np.ndarray, **kwargs):
    c = np.ascontiguousarray(
        np.asarray(c_embeds, dtype=np.float32).reshape(B, N, E)
    )
    s = np.ascontiguousarray(np.asarray(s_embeds, dtype=np.float32))
    nc = _get_nc()
    in_maps = [
        {
            "c": c[k * B_LOC:(k + 1) * B_LOC],
            "s": s[k * B_LOC:(k + 1) * B_LOC],
        }
        for k in range(NCORES)
    ]
    r = run_bass_kernel_spmd(nc, in_maps, core_ids=list(range(NCORES)), **kwargs)
    # o[p, (b, g, j)] -> out[b, g*GROUP_N + p*ROWS + j]
    parts = []
    for k in range(NCORES):
        o = r.results[k]["o"].reshape(P, B_LOC, G, ROWS)
        parts.append(
            np.ascontiguousarray(o.transpose(1, 2, 0, 3)).reshape(B_LOC, N)
        )
    out = np.concatenate(parts, axis=0)
    return out.astype(np.float32), r


def kernel(c_embeds: np.ndarray, s_embeds: np.ndarray) -> np.ndarray:
    out, _ = _run(c_embeds, s_embeds)
    return out


# revision 24
# speedup vs baseline: 1.0666x; 1.0666x over previous
"""Context-segment scoring kernel for Trainium2 (Bass/Tile).

Computes out[b, n] = sum_e c[b, n, e] * s[b, e] for
c = c_embeds [32, 32, 32, 8, 256] viewed as [B=32, N=8192, E=256] and
s = s_embeds [32, 256].

Sharding: data-parallel over batch — 8 NeuronCores, 4 batches each.
Per core: stream c (32 MiB) through SBUF in 2 MiB groups
([128 partitions x 16 rows x 256]) on the Sync-engine HWDGE ring,
loaded as halves (quarters at the pipeline edges) so completion
granularity is fine enough for buffer recycling to never gate a
straggling SDMA engine; cin lookahead is 11 groups (22 MiB).
The per-batch segment embeddings are broadcast on-chip: one 4 KiB DMA
stages s on a single partition, the idle TensorE multiplies it by a
ones[1,128] stationary to replicate it across partitions into PSUM,
and the DVE reads in1 straight from PSUM — no 128x-amplified HBM
broadcast reads. Reduce work splits 10:6 between DVE
(tensor_tensor_reduce rows, in place) and ScalarE (activation accum
reduces of a wide DVE product). Results accumulate in one resident
[128, 256] tile whose free-dim layout (b, g, j) makes store
descriptors contiguous per partition; batches store as they finish and
the final 8 KiB piece goes out on the idle Sync ring. The host
transposes the permuted [128, 256] DRAM image back to [4, 8192].
"""

import numpy as np

import concourse.bacc as bacc
import concourse.bass as bass
import concourse.mybir as mybir
import concourse.tile as tile
from concourse.bass_utils import run_bass_kernel_spmd

B, N, E = 32, 8192, 256
NCORES = 8
B_LOC = B // NCORES          # 4 batches per core
P = 128                      # SBUF partitions
ROWS = 16                    # n-rows per partition per group
GROUP_N = P * ROWS           # 2048 n per group
G = N // GROUP_N             # 4 groups per batch
NGROUPS = G * B_LOC          # 16 groups per core
# Per-group engine plan: 'A' = fused multiply+reduce rows on DVE (in-place,
# no product tile); 'S' = wide DVE multiplies (halves), ScalarE reduces the
# rows. GpSimd elementwise is NOT used: it share-locks the DVE SBUF port and
# was measured to slow every concurrent DVE op by ~36%.
# 'A' = per-row fused multiply+reduce on DVE (~6.6us/group, single pass at
# the DVE reduce rate); 'S' = wide DVE multiplies (halves, fast elementwise
# rate) + ScalarE activation-accum rows (~3.1us DVE + ~9.3us Scalar).
# 9A:7S balances DVE ~81us / Scalar ~77us against the ~83us DMA stream.
# ('R' whole-group tensor_reduce was measured WORSE: standalone reduce runs
# at ~1.7ns/elem, half the elementwise rate -> 10us/group all-DVE.)
PLAN = ["A", "S", "A", "S", "A", "S", "A", "S", "A", "S", "A", "S", "S", "A", "A", "A"]

# HW-validated flags (tensor_tensor_reduce in-place dies on HW; avoid)
USE_PE_BCAST = True   # s via TensorE broadcast into PSUM (False: HBM bcast DMA)
USE_SYNC_STORE = True  # final 8 KiB store on Sync ring (False: Scalar)

F32 = mybir.dt.float32


def build_body(tc, out_ap, c_ap, s_ap):
    """Trace the per-core Tile program. APs are DRAM access patterns:
    out [P, B_LOC*G*ROWS] (permuted; host untangles), c [B_LOC, N, E],
    s [B_LOC, E]."""
    nc = tc.nc
    with (
        tc.tile_pool(name="sstage", bufs=1) as sstage_pool,
        tc.tile_pool(name="ones", bufs=1) as ones_pool,
        tc.tile_pool(name="sps", bufs=1, space="PSUM") as sps_pool,
        tc.tile_pool(name="cpair", bufs=4) as cpair_pool,
        tc.tile_pool(name="cedge", bufs=2) as cedge_pool,
        tc.tile_pool(name="prod", bufs=2) as prod_pool,
        tc.tile_pool(name="res", bufs=1) as res_pool,
        tc.tile_pool(name="dump", bufs=2) as dump_pool,
    ):
        # Flat n view so load pairs can cross batch boundaries.
        c_flat = c_ap.rearrange("b n e -> (b n) e")

        # Loads: the Tile scheduler tracks DMA completion on only 8 sem
        # lanes per issuing engine, so at most 8 dma_starts are in flight
        # per ring. Middle groups therefore load as PAIRS (one 4 MiB
        # dispatch, 32 KiB/partition descriptors) so the window covers the
        # whole stream and the SBUF pool (4 pair bufs + edges ~ 10 groups)
        # is the only recycle gate. The first group loads as quarters on
        # the then-idle Scalar ring (DVE starts sooner, Sync's window is
        # spent on pairs); the last group as quarters on Sync so the
        # compute tail stays short.
        tiles = {}

        def load(gi):
            if gi in (0, NGROUPS - 1):
                ct = cedge_pool.tile([P, ROWS, E], F32, tag="cedge", name="ce")
                src = c_flat[gi * GROUP_N:(gi + 1) * GROUP_N, :].rearrange(
                    "(p j) e -> p j e", j=ROWS
                )
                eng = nc.scalar if gi == 0 else nc.sync
                C = ROWS // 4
                for q in range(4):
                    eng.dma_start(
                        ct[:, q * C:(q + 1) * C, :], src[:, q * C:(q + 1) * C, :]
                    )
                tiles[gi] = ct
            else:  # odd gi: load the (gi, gi+1) pair in one dispatch.
                # The 4-D structure lives only in the DRAM-side AP (each
                # partition reads two 16 KiB chunks, one per group); the
                # SBUF tile stays 3-D contiguous so compute slices have
                # the exact same access patterns as single-group tiles.
                cp = cpair_pool.tile([P, 2 * ROWS, E], F32, tag="cpair", name="cp")
                src = c_flat[gi * GROUP_N:(gi + 2) * GROUP_N, :].rearrange(
                    "(gg p j) e -> p gg j e", gg=2, j=ROWS
                )
                dst = cp[:, :, :].rearrange("p (gg j) e -> p gg j e", gg=2)
                nc.sync.dma_start(dst, src)
                tiles[gi] = cp[:, 0:ROWS, :]
                tiles[gi + 1] = cp[:, ROWS:2 * ROWS, :]

        # --- on-chip segment-embedding broadcast -------------------------
        # One 4 KiB DMA lands all four batches' s on partition 0; TensorE
        # replicates each across 128 partitions into its own PSUM bank
        # (ones[1,128].T @ s[1,256]); ScalarE copies each into SBUF.
        if USE_PE_BCAST:
            s_stage = sstage_pool.tile([1, B_LOC * E], F32, tag="s_stage")
            nc.scalar.dma_start(
                s_stage[:, :], s_ap.rearrange("b e -> (b e)").unsqueeze(0)
            )
            # Group 0's quarters go out on Scalar right behind the tiny
            # s_stage load, BEFORE the PSUM->SBUF copies (which block on
            # the PE matmuls) so the c stream starts immediately.
            load(0)
            ones = ones_pool.tile([1, P], F32, tag="ones")
            nc.vector.memset(ones[:, :], 1.0)
            # one PSUM bank (512 f32) per batch so each matmul output is
            # bank-aligned
            s_ps = sps_pool.tile([P, B_LOC, 512], F32, tag="s_ps")
            for b in range(B_LOC):
                nc.tensor.matmul(
                    s_ps[:, b, 0:E],
                    ones[:, :],
                    s_stage[:, b * E:(b + 1) * E],
                    start=True,
                    stop=True,
                )
            # ScalarE copies each batch's broadcast PSUM->SBUF: DVE rows
            # read in1 from SBUF at full rate (PSUM in1 cost ~+25% per row).
            s_sb = ones_pool.tile([P, B_LOC * E], F32, tag="s_sb")
            for b in range(B_LOC):
                nc.scalar.copy(s_sb[:, b * E:(b + 1) * E], s_ps[:, b, 0:E])
            s_in1 = [s_sb[:, b * E:(b + 1) * E] for b in range(B_LOC)]
        else:
            sb_all = sstage_pool.tile([P, B_LOC * E], F32, tag="s_sb")
            load(0)
            for b in range(B_LOC):
                s_src = s_ap[b, :].unsqueeze(0).broadcast_to([P, E])
                nc.scalar.dma_start(sb_all[:, b * E:(b + 1) * E], s_src)
            s_in1 = [sb_all[:, b * E:(b + 1) * E] for b in range(B_LOC)]

        # All per-row results accumulate in one SBUF tile; free-dim order
        # (b, g, j) keeps each store's per-partition bytes contiguous.
        res_all = res_pool.tile([P, B_LOC, G, ROWS], F32, tag="res")

        HALF = ROWS // 2

        for b in range(B_LOC):
            for g in range(G):
                gi = b * G + g
                if gi not in tiles:
                    load(gi)
                ct = tiles.pop(gi)

                res = res_all[:, b, g, :]
                if PLAN[gi] == "A":
                    # Fused multiply+reduce per row on DVE: fine-grained so
                    # the first group computes per-quarter as data lands and
                    # the last group's tail is short. In-place over ct.
                    for j in range(ROWS):
                        nc.vector.affine_mul_reduce(
                            out=ct[:, j, :],
                            accum_out=res[:, j:j + 1],
                            in0=ct[:, j, :],
                            in1=s_in1[b],
                            scale=1.0,
                            bias=0.0,
                        )
                elif PLAN[gi] == "R":
                    # Whole-group path, all DVE, two wide ops: in-place
                    # multiply, then one segmented reduce over the innermost
                    # axis ([P,16,256] -> [P,16]). Avoids the ~230 ns
                    # per-instruction overhead of 16 row ops.
                    s_bc = s_in1[b].unsqueeze(1).broadcast_to([P, ROWS, E])
                    nc.vector.tensor_tensor(
                        out=ct[:],
                        in0=ct[:],
                        in1=s_bc,
                        op=mybir.AluOpType.mult,
                    )
                    nc.vector.tensor_reduce(
                        out=res[:, :],
                        in_=ct[:],
                        axis=mybir.AxisListType.X,
                        op=mybir.AluOpType.add,
                    )
                else:
                    # Two wide DVE multiplies (halves, so the product pool
                    # recycles finely), then ScalarE reduces the rows.
                    for h in range(2):
                        pr = prod_pool.tile([P, HALF, E], F32, tag="prod", name="pr")
                        s_bc = s_in1[b].unsqueeze(1).broadcast_to([P, HALF, E])
                        nc.vector.tensor_tensor(
                            out=pr[:],
                            in0=ct[:, h * HALF:(h + 1) * HALF, :],
                            in1=s_bc,
                            op=mybir.AluOpType.mult,
                        )
                        dump = dump_pool.tile([P, E], F32, tag="dump", name="dump")
                        for j in range(HALF):
                            nc.scalar.activation(
                                dump[:, :],
                                pr[:, j, :],
                                mybir.ActivationFunctionType.Copy,
                                bias=0.0,
                                scale=1.0,
                                accum_out=res[:, h * HALF + j:h * HALF + j + 1],
                            )

            # Store finished results eagerly so only the last (8 KiB) piece
            # sits on the critical-path tail. res free-dim layout makes each
            # store's bytes contiguous per partition (>=256 B descriptors).
            fb = b * G * ROWS
            if b < B_LOC - 1:
                nc.scalar.dma_start(
                    out_ap[:, fb:fb + G * ROWS], res_all[:, b, :, :]
                )
        # Last batch: groups 0-2 store from the ScalarE ring as soon as
        # group 2 is done; the final group's 8 KiB goes out on the idle
        # Sync ring right after its last row completes.
        lb = (B_LOC - 1) * G * ROWS
        nc.scalar.dma_start(
            out_ap[:, lb:lb + 3 * ROWS], res_all[:, B_LOC - 1, 0:3, :]
        )
        eng = nc.sync if USE_SYNC_STORE else nc.scalar
        eng.dma_start(
            out_ap[:, lb + 3 * ROWS:lb + 4 * ROWS],
            res_all[:, B_LOC - 1, 3, :],
        )


_NC_CACHE = None


def _get_nc():
    global _NC_CACHE
    if _NC_CACHE is None:
        nc = bacc.Bacc(
            "TRN2",
            target_bir_lowering=False,
            debug=False,
            num_devices=NCORES,
        )
        c = nc.dram_tensor("c", [B_LOC, N, E], F32, kind="ExternalInput")
        s = nc.dram_tensor("s", [B_LOC, E], F32, kind="ExternalInput")
        o = nc.dram_tensor("o", [P, B_LOC * G * ROWS], F32, kind="ExternalOutput")
        with tile.TileContext(nc) as tc:
            build_body(tc, o.ap(), c.ap(), s.ap())
        nc.compile()
        _NC_CACHE = nc
    return _NC_CACHE


def _run(c_embeds: np.ndarray, s_embeds: np.ndarray, **kwargs):
    c = np.ascontiguousarray(
        np.asarray(c_embeds, dtype=np.float32).reshape(B, N, E)
    )
    s = np.ascontiguousarray(np.asarray(s_embeds, dtype=np.float32))
    nc = _get_nc()
    in_maps = [
        {
            "c": c[k * B_LOC:(k + 1) * B_LOC],
            "s": s[k * B_LOC:(k + 1) * B_LOC],
        }
        for k in range(NCORES)
    ]
    r = run_bass_kernel_spmd(nc, in_maps, core_ids=list(range(NCORES)), **kwargs)
    # o[p, (b, g, j)] -> out[b, g*GROUP_N + p*ROWS + j]
    parts = []
    for k in range(NCORES):
        o = r.results[k]["o"].reshape(P, B_LOC, G, ROWS)
        parts.append(
            np.ascontiguousarray(o.transpose(1, 2, 0, 3)).reshape(B_LOC, N)
        )
    out = np.concatenate(parts, axis=0)
    return out.astype(np.float32), r


def kernel(c_embeds: np.ndarray, s_embeds: np.ndarray) -> np.ndarray:
    out, _ = _run(c_embeds, s_embeds)
    return out
